# revision 2
# baseline (speedup 1.0000x reference)
"""Trainium2 Bass kernel for DeepSeek-style MoE gate routing (v2).

hidden_states [8, 4096, 2048] f32, w [256, 2048] f32, bias [256] f32
 -> topk_idx [32768, 8] int32, topk_weight [32768, 8] f32

Tokens split 8 ways across NeuronCores (4096/core); gate weight + bias
replicated.  x is host-packed so the hidden dim lands on SBUF partitions
with fully-contiguous per-group DMA.

Matmul modes:
  mix2b    - lhsT = x fp32 (stationary, exact), rhs = [w_hi | w_lo] bf16
             as ONE N=512 moving operand; halves folded with a DVE add.
             2 effective passes, fp32-grade logits.
  mix2a    - same split but two N=256 matmuls accumulating in PSUM.
  f32r_1p  - single fp32r pass (both operands rounded to ~12 bits by HW).
  split3   - baseline 3-pass bf16 scheme (x hi/lo + w hi/lo).

Top-k section (all modes): group top-2 via match_replace, group mask via
rank counting, top-8 via MAX8/FIND_INDEX8; weights recovered as
v8 - bias[idx] with a gpsimd indirect_copy gather (no sigma embedding).

Self-contained: hardcodes all shapes; only imports the concourse toolchain.
"""
import sys

if "/opt/trn_rl_repo" not in sys.path:
    sys.path.insert(0, "/opt/trn_rl_repo")

import numpy as np

import concourse.bass as bass  # noqa: F401
import concourse.mybir as mybir
import concourse.tile as tile
from concourse import bacc
from concourse.bass_utils import run_bass_kernel_spmd

P = 128            # partitions / tokens per tile
H = 2048           # hidden dim
E = 256            # experts
KO = H // P        # 16 contraction chunks
N_CORES = 8
T_CORE = 4096      # tokens per core
N_TILES = T_CORE // P       # 32 token tiles per core

N_GROUP = 8
GSIZE = E // N_GROUP        # 32
TOPK_GROUP = 4
TOP_K = 8
SCALING = 2.5
NEG_BIG = -1.0e30

MATMUL_MODE = "split3"

GROUPS = [1, 3, 4, 4, 4, 4, 4, 4, 2, 2]
assert sum(GROUPS) == N_TILES
N_WARM = 10        # dummy matmuls to pre-warm the PE HAM clock gate

f32 = mybir.dt.float32
f32r = mybir.dt.float32r
f16 = mybir.dt.float16
bf16 = mybir.dt.bfloat16
u32 = mybir.dt.uint32
u16 = mybir.dt.uint16
ALU = mybir.AluOpType
ACTF = mybir.ActivationFunctionType
AX = mybir.AxisListType

_CACHED_NC = {}


def build_kernel(mode=MATMUL_MODE):
    nc = bacc.Bacc("TRN2", target_bir_lowering=False, debug=False)

    if mode in ("mix2b", "mix2a"):
        d_x = [nc.dram_tensor("xp", [H * T_CORE], f32, kind="ExternalInput")]
        d_w = nc.dram_tensor("wp", [P, KO, 2 * E], bf16, kind="ExternalInput")
        xdt, wdt, wfree = f32, bf16, 2 * E
    elif mode == "f32r_1p":
        d_x = [nc.dram_tensor("xp", [H * T_CORE], f32r, kind="ExternalInput")]
        d_w = nc.dram_tensor("wp", [P, KO, E], f32r, kind="ExternalInput")
        xdt, wdt, wfree = f32r, f32r, E
    elif mode == "split3":
        d_x = [nc.dram_tensor("xph", [H * T_CORE], bf16, kind="ExternalInput"),
               nc.dram_tensor("xpl", [H * T_CORE], bf16, kind="ExternalInput")]
        d_w = nc.dram_tensor("wp", [P, KO, 2 * E], bf16, kind="ExternalInput")
        xdt, wdt, wfree = bf16, bf16, 2 * E
    else:
        raise ValueError(mode)
    d_bias = nc.dram_tensor("biasrep", [P, E], f32, kind="ExternalInput")
    d_oidx = nc.dram_tensor("oidx", [P, N_TILES, TOP_K], u16, kind="ExternalOutput")
    d_owgt = nc.dram_tensor("owgt", [P, N_TILES, TOP_K], f32, kind="ExternalOutput")

    psum_free = 2 * E if mode == "mix2b" else E

    with tile.TileContext(nc) as tc:
        with tc.tile_pool(name="const", bufs=1) as cpool, \
             tc.tile_pool(name="xin", bufs=3) as xpool, \
             tc.tile_pool(name="score", bufs=2) as spool, \
             tc.tile_pool(name="small", bufs=3) as mpool, \
             tc.tile_pool(name="psum", bufs=6, space="PSUM") as ppool, \
             tc.tile_pool(name="psumw", bufs=2, space="PSUM") as wpool:

            # ---- constants ----
            # two independent halves so k<8 matmuls need only the first
            whl_a = cpool.tile([P, KO // 2, wfree], wdt, name="whl_a")
            whl_b = cpool.tile([P, KO // 2, wfree], wdt, name="whl_b")
            nc.sync.dma_start(whl_a, d_w.ap()[:, :KO // 2])
            nc.sync.dma_start(whl_b, d_w.ap()[:, KO // 2:])

            def wsl(k):
                return (whl_a, k) if k < KO // 2 else (whl_b, k - KO // 2)

            bias_sb = cpool.tile([P, E], f32)
            nc.sync.dma_start(bias_sb, d_bias.ap())
            negbig = cpool.tile([P, 1], f32)
            nc.vector.memset(negbig, NEG_BIG)
            oidx_sb = cpool.tile([P, N_TILES, TOP_K], u16)
            owgt_sb = cpool.tile([P, N_TILES, TOP_K], f32)

            # warm the PE HAM clock gate with throwaway matmuls while the
            # first w/x DMAs are in flight (garbage operands, result unread)
            scrA = cpool.tile([P, P], bf16)
            scrB = cpool.tile([P, 2 * E], bf16)
            nc.vector.memset(scrA, 0.0)
            nc.vector.memset(scrB, 0.0)
            for _ in range(N_WARM):
                pw = wpool.tile([P, 2 * E], f32, tag="pw")
                nc.tensor.matmul(pw, lhsT=scrA, rhs=scrB, start=True, stop=True,
                                 skip_group_check=True)

            tl0 = 0
            for nt in GROUPS:
                stok = nt * P
                x_sb = []
                off = tl0 * P * H
                for i, d in enumerate(d_x):
                    t = xpool.tile([P, KO, stok], xdt, tag=f"x{i}")
                    nc.sync.dma_start(
                        t, d.ap()[off:off + P * KO * stok]
                            .rearrange("(p ko t) -> p ko t", p=P, ko=KO))
                    x_sb.append(t)

                sb_st = spool.tile([P, nt, E], f32, tag="sb")
                zap_st = spool.tile([P, nt, E], f32, tag="zap")
                msf_st = spool.tile([P, nt, E], f32, tag="msf")
                t1g = mpool.tile([P, nt, N_GROUP], f32, tag="t1g")
                t2g = mpool.tile([P, nt, N_GROUP], f32, tag="t2g")
                gs = mpool.tile([P, nt, N_GROUP], f32, tag="gs")
                cc = mpool.tile([P, nt, N_GROUP, N_GROUP], f32, tag="cc")
                c8 = mpool.tile([P, nt, N_GROUP], f32, tag="c8")
                madd = mpool.tile([P, nt, N_GROUP], f32, tag="madd")
                v8 = mpool.tile([P, nt, 8], f32, tag="v8")
                bias8 = mpool.tile([P, nt, 8], f32, tag="bias8")
                w8 = mpool.tile([P, nt, 8], f32, tag="w8")
                ssum = mpool.tile([P, nt, 1], f32, tag="ssum")
                rs = mpool.tile([P, nt, 1], f32, tag="rs")

                for j in range(nt):
                    tl = tl0 + j
                    tsl = slice(j * P, (j + 1) * P)

                    # ---- logits ----
                    ps = ppool.tile([P, psum_free], f32, tag="ps")
                    if mode == "mix2b":
                        for k in range(KO):
                            wt, kk = wsl(k)
                            nc.tensor.matmul(
                                ps, lhsT=x_sb[0][:, k, tsl], rhs=wt[:, kk, :],
                                start=(k == 0), stop=(k == KO - 1))
                        # fold hi|lo halves (PSUM -> SBUF), then sigmoid+bias
                        lsum = spool.tile([P, E], f32, tag="lsum")
                        nc.vector.tensor_add(lsum, ps[:, :E], ps[:, E:])
                        sig_src = lsum
                    elif mode == "mix2a":
                        for k in range(KO):
                            wt, kk = wsl(k)
                            nc.tensor.matmul(
                                ps, lhsT=x_sb[0][:, k, tsl], rhs=wt[:, kk, :E],
                                start=(k == 0), stop=False)
                            nc.tensor.matmul(
                                ps, lhsT=x_sb[0][:, k, tsl], rhs=wt[:, kk, E:],
                                start=False, stop=(k == KO - 1))
                        sig_src = ps
                    elif mode == "f32r_1p":
                        for k in range(KO):
                            wt, kk = wsl(k)
                            nc.tensor.matmul(
                                ps, lhsT=x_sb[0][:, k, tsl], rhs=wt[:, kk, :],
                                start=(k == 0), stop=(k == KO - 1))
                        sig_src = ps
                    else:  # split3
                        xh, xl = x_sb
                        for k in range(KO):
                            wt, kk = wsl(k)
                            nc.tensor.matmul(
                                ps, lhsT=xh[:, k, tsl], rhs=wt[:, kk, :E],
                                start=(k == 0), stop=False)
                            nc.tensor.matmul(
                                ps, lhsT=xh[:, k, tsl], rhs=wt[:, kk, E:],
                                start=False, stop=False)
                            nc.tensor.matmul(
                                ps, lhsT=xl[:, k, tsl], rhs=wt[:, kk, :E],
                                start=False, stop=(k == KO - 1))
                        sig_src = ps

                    # ---- sigma = sigmoid(logits) on ACT ----
                    nc.scalar.activation(sb_st[:, j, :], sig_src, ACTF.Sigmoid)

                # scores_for_choice = sigma + bias (batched, DVE)
                nc.vector.tensor_add(
                    sb_st, sb_st, bias_sb[:, None, :].to_broadcast([P, nt, E]))

                # ---- group top-2 (batched reduce + per-tile match_replace) ----
                sb4 = sb_st.rearrange("p t (g e) -> p t g e", g=N_GROUP)
                nc.vector.tensor_reduce(out=t1g, in_=sb4, axis=AX.X, op=ALU.max)
                for j in range(nt):
                    nc.vector.match_replace(
                        out=zap_st[:, j, :], in_to_replace=t1g[:, j, :],
                        in_values=sb_st[:, j, :], imm_value=NEG_BIG)
                nc.vector.tensor_reduce(
                    out=t2g, in_=zap_st.rearrange("p t (g e) -> p t g e", g=N_GROUP),
                    axis=AX.X, op=ALU.max)
                nc.vector.tensor_add(gs, t1g, t2g)

                # ---- group rank count + additive mask ----
                nc.vector.tensor_tensor(
                    out=cc,
                    in0=gs[:, :, None, :].to_broadcast([P, nt, N_GROUP, N_GROUP]),
                    in1=gs[:, :, :, None].to_broadcast([P, nt, N_GROUP, N_GROUP]),
                    op=ALU.is_gt)
                nc.vector.tensor_reduce(out=c8, in_=cc, axis=AX.X, op=ALU.add)
                nc.vector.scalar_tensor_tensor(
                    madd, c8, float(TOPK_GROUP) - 0.5,
                    negbig[:, :, None].to_broadcast([P, nt, N_GROUP]),
                    op0=ALU.is_gt, op1=ALU.mult)

                # ---- masked scores ----
                nc.vector.tensor_add(
                    msf_st.rearrange("p t (g e) -> p t g e", g=N_GROUP),
                    sb4,
                    madd[:, :, :, None].to_broadcast([P, nt, N_GROUP, GSIZE]))

                # ---- top-8 of masked scores ----
                # owgt carries the raw top-8 score values (sigma + bias);
                # the tiny bias-subtract + normalize epilogue runs on host.
                for j in range(nt):
                    tl = tl0 + j
                    nc.vector.max(out=owgt_sb[:, tl, :], in_=msf_st[:, j, :])
                    nc.vector.max_index(out=oidx_sb[:, tl, :],
                                        in_max=owgt_sb[:, tl, :],
                                        in_values=msf_st[:, j, :])

                ssl = slice(tl0, tl0 + nt)
                nc.scalar.dma_start(d_oidx.ap()[:, ssl, :], oidx_sb[:, ssl, :])
                nc.scalar.dma_start(d_owgt.ap()[:, ssl, :], owgt_sb[:, ssl, :])
                tl0 += nt

    nc.compile()
    return nc


def _get_nc(mode):
    if mode not in _CACHED_NC:
        _CACHED_NC[mode] = build_kernel(mode)
    return _CACHED_NC[mode]


def _pack_x(xTc):
    """[H, T_CORE] -> packed 1D so each group's DMA is fully contiguous.

    Block for group (tl0, nt): [P, KO, nt*P] with [p, ko, t] = xTc[ko*P+p, tl0*P+t].
    """
    arr = xTc.reshape(KO, P, T_CORE)
    blocks = []
    tl0 = 0
    for nt in GROUPS:
        blocks.append(np.ascontiguousarray(
            arr[:, :, tl0 * P:(tl0 + nt) * P].transpose(1, 0, 2)).reshape(-1))
        tl0 += nt
    return np.concatenate(blocks)


def _pack_w(wTp):
    """[H, E] -> [P, KO, E] with [p, ko, e] = wTp[ko*P+p, e]."""
    return np.ascontiguousarray(wTp.reshape(KO, P, E).transpose(1, 0, 2))


def kernel(hidden_states, w, e_score_correction_bias, mode=MATMUL_MODE):
    hidden_states = np.asarray(hidden_states)
    w = np.asarray(w)
    e_score_correction_bias = np.asarray(e_score_correction_bias)
    T = hidden_states.shape[0] * hidden_states.shape[1]
    assert T == N_CORES * T_CORE
    x2 = np.ascontiguousarray(hidden_states.reshape(T, H).astype(np.float32))
    xT = np.ascontiguousarray(x2.T)                         # [H, T]
    wT = np.ascontiguousarray(np.asarray(w, np.float32).T)  # [H, E]
    bias_rep = np.ascontiguousarray(
        np.repeat(np.asarray(e_score_correction_bias, np.float32)[None, :], P, 0))

    import ml_dtypes
    bf = ml_dtypes.bfloat16
    if mode in ("mix2b", "mix2a"):
        xparts = {"xp": xT}
        wh = _pack_w(wT.astype(bf))
        wl = _pack_w((wT - wT.astype(bf).astype(np.float32)).astype(bf))
        wp = np.ascontiguousarray(np.concatenate([wh, wl], axis=2))
    elif mode == "f32r_1p":
        xparts = {"xp": xT}
        wp = _pack_w(wT)
    else:  # split3
        xh = xT.astype(bf)
        xl = (xT - xh.astype(np.float32)).astype(bf)
        xparts = {"xph": xh, "xpl": xl}
        wh = _pack_w(wT.astype(bf))
        wl = _pack_w((wT - wT.astype(bf).astype(np.float32)).astype(bf))
        wp = np.ascontiguousarray(np.concatenate([wh, wl], axis=2))

    nc = _get_nc(mode)
    in_maps = []
    for c in range(N_CORES):
        m = {k: _pack_x(v[:, c * T_CORE:(c + 1) * T_CORE])
             for k, v in xparts.items()}
        m["wp"] = wp
        m["biasrep"] = bias_rep
        in_maps.append(m)

    res = run_bass_kernel_spmd(nc, in_maps, core_ids=list(range(N_CORES)))

    idx_parts, wgt_parts = [], []
    for c in range(N_CORES):
        r = res.results[c]
        idx_parts.append(r["oidx"].transpose(1, 0, 2).reshape(T_CORE, TOP_K))
        wgt_parts.append(r["owgt"].transpose(1, 0, 2).reshape(T_CORE, TOP_K))
    topk_idx = np.concatenate(idx_parts, 0).astype(np.int32)
    v8 = np.concatenate(wgt_parts, 0).astype(np.float32)
    # epilogue: v8 holds the top-8 (sigma + bias) values; recover sigma,
    # normalize, scale (cheap elementwise work on [T, 8])
    bias_f = np.asarray(e_score_correction_bias, np.float32)
    sigma = v8 - bias_f[topk_idx]
    topk_weight = (sigma / sigma.sum(-1, keepdims=True) * SCALING).astype(np.float32)
    return topk_idx, topk_weight
